# revision 14
# baseline (speedup 1.0000x reference)
"""Trainium2 Bass kernel for nn_DecoderLayer (post-LN decoder layer).

Sharding: data-parallel over batch. B=8 batch elements -> 8 NeuronCores,
one full decoder layer per core, zero collectives.

Per-core strategy (v2 — fp8 + token-half pipelining):
  - All six attention projections (Q/K/V x self/cross) run in fp8-e4m3 with
    the DoubleRow perf mode (2 contraction tiles per instruction, 2x PE
    throughput). Weights are pre-scaled x64 on the host so w-values clear
    the e4m3 denormal floor; evictions fold the 1/64 back in. Attention
    error is damped by the residual stream (|o| << |x|), so fp8 there is
    cheap accuracy-wise; the FFN and all LayerNorm/residual math stay bf16+.
  - exp() is written straight to fp8 by ScalarE; AV runs as fp8 DoubleRow
    over m-chunk pairs with the [v | 1] ones-column trick for sum(exp).
    Scores stay bf16 (d_head=64 contraction can't pair k-tiles).
  - The layer is pipelined over token halves (512 tokens): attention1 ->
    LN1 -> Q2 per half, then attention2 -> LN2 -> FFN -> LN3 per half.
    Half 0's FFN matmuls (PE) overlap half 1's exp (ACT), which is the
    attention-phase bottleneck. The Tile scheduler finds the overlap from
    per-chunk data deps; all PSUM/SBUF pools are top-level so no scope
    barriers get in the way.
  - Engine pinning: ACT runs only exp + LN rstd (one activation table set:
    natural_log_exp covers Exp+Ln). DVE takes all PSUM evictions. Pool
    (GpSimd) takes the SBUF-only LN residual adds and LN applies, plus the
    v65 memsets. PE p-state is kept warm by always having scores/AV/proj/
    FFN matmuls ready.
  - LayerNorm rstd is batched per half (one Ln+Exp pair per 4 chunks);
    x1/x2 stay SBUF-resident (no DRAM spill). Host folds biases as before:
    x_tok = x + bv1; be1' = be1 + bv2; bq2' = bq2 - Wq2 @ bv2;
    be2' = be2 + b2; b1' = b1 - W1 @ b2.
"""

import os
import numpy as np
import ml_dtypes

BF16 = ml_dtypes.bfloat16
FP8 = ml_dtypes.float8_e4m3fn

D = 1024
N = 1024
H = 16
DK = 64
FF = 4096
P = 128
DC = D // P     # 8 feature chunks
NC_ = N // P    # 8 token chunks
NH = 2          # token halves
HL = N // NH    # 512 tokens per half
HC = NC_ // NH  # 4 chunks per half
EPS = 1e-5
N_CORES = 8
WS = 64.0       # host-side fp8 weight pre-scale
WSI = 1.0 / WS

_BUILD_CACHE = {}


def _build_program(loop_n=1, ln_identity=False):
    key = (loop_n, ln_identity)
    if key in _BUILD_CACHE:
        return _BUILD_CACHE[key]

    from contextlib import ExitStack

    import concourse.bass as bass
    import concourse.mybir as mybir
    import concourse.tile as tile
    from concourse import bacc
    from concourse.masks import make_identity

    dt = mybir.dt
    AF = mybir.ActivationFunctionType
    ALU = mybir.AluOpType
    PM = mybir.MatmulPerfMode

    nc = bacc.Bacc("TRN2", target_bir_lowering=False, debug=False)

    # ---- DRAM parameters (per core) ----
    x_tok_d = nc.dram_tensor("x_tok", [N, D], dt.bfloat16, kind="ExternalInput")
    xT8_d = nc.dram_tensor("xT8", [D, N], dt.float8e4, kind="ExternalInput")
    memT8_d = nc.dram_tensor("memT8", [D, N], dt.float8e4, kind="ExternalInput")
    w8_d = {}
    for nm in ("q1", "k1", "v1", "q2", "k2", "v2"):
        w8_d[nm] = nc.dram_tensor(f"W{nm}T8", [D, D], dt.float8e4,
                                  kind="ExternalInput")
    W1T_d = nc.dram_tensor("W1T", [D, FF], dt.bfloat16, kind="ExternalInput")
    W2T_d = nc.dram_tensor("W2T", [FF, D], dt.bfloat16, kind="ExternalInput")
    bq1_d = nc.dram_tensor("bq1", [D], dt.float32, kind="ExternalInput")
    bk1_d = nc.dram_tensor("bk1", [D], dt.float32, kind="ExternalInput")
    bq2_d = nc.dram_tensor("bq2p", [D], dt.float32, kind="ExternalInput")
    bk2_d = nc.dram_tensor("bk2", [D], dt.float32, kind="ExternalInput")
    b1_d = nc.dram_tensor("b1p", [FF], dt.float32, kind="ExternalInput")
    g1_d = nc.dram_tensor("g1", [D], dt.bfloat16, kind="ExternalInput")
    be1_d = nc.dram_tensor("be1p", [D], dt.bfloat16, kind="ExternalInput")
    g2_d = nc.dram_tensor("g2", [D], dt.bfloat16, kind="ExternalInput")
    be2_d = nc.dram_tensor("be2p", [D], dt.bfloat16, kind="ExternalInput")
    g3_d = nc.dram_tensor("g3", [D], dt.bfloat16, kind="ExternalInput")
    be3_d = nc.dram_tensor("be3", [D], dt.bfloat16, kind="ExternalInput")
    out_d = nc.dram_tensor("out", [N, D], dt.float32, kind="ExternalOutput")

    def bcast_ap(handle, n):
        return bass.AP(tensor=handle, offset=0, ap=[[0, P], [1, n]])

    def colmajor_ap(handle, chunks):
        return bass.AP(tensor=handle, offset=0, ap=[[1, P], [P, chunks]])

    with tile.TileContext(nc) as tc, ExitStack() as top:
        consts = top.enter_context(tc.tile_pool(name="consts", bufs=1))
        qkp = top.enter_context(tc.tile_pool(name="qk", bufs=1))
        vp = top.enter_context(tc.tile_pool(name="vp", bufs=1))
        resp = top.enter_context(tc.tile_pool(name="res", bufs=2))
        ep = top.enter_context(tc.tile_pool(name="ep", bufs=2))
        op_ = top.enter_context(tc.tile_pool(name="op", bufs=2))
        small = top.enter_context(tc.tile_pool(name="small", bufs=8))
        psp = top.enter_context(tc.tile_pool(name="ps", bufs=2, space="PSUM"))

        # ---- constants ----
        bq1_sb = consts.tile([P, DC], dt.float32, tag="c_bq1")
        nc.sync.dma_start(out=bq1_sb, in_=colmajor_ap(bq1_d, DC))
        bk1_sb = consts.tile([P, DC], dt.float32, tag="c_bk1")
        nc.sync.dma_start(out=bk1_sb, in_=colmajor_ap(bk1_d, DC))
        bq2_sb = consts.tile([P, DC], dt.float32, tag="c_bq2")
        nc.sync.dma_start(out=bq2_sb, in_=colmajor_ap(bq2_d, DC))
        bk2_sb = consts.tile([P, DC], dt.float32, tag="c_bk2")
        nc.sync.dma_start(out=bk2_sb, in_=colmajor_ap(bk2_d, DC))
        b1_sb = consts.tile([P, FF // P], dt.float32, tag="c_b1")
        nc.sync.dma_start(out=b1_sb, in_=colmajor_ap(b1_d, FF // P))
        eps_sb = consts.tile([P, 1], dt.float32, tag="c_eps")
        nc.vector.memset(eps_sb, EPS)
        ident_sb = consts.tile([P, P], dt.bfloat16, tag="c_ident")
        make_identity(nc, ident_sb)

        def load_w8(pool, dram_h, name):
            """fp8 [D, D] weight -> SBUF [P, DC, D], per-k-chunk DMA."""
            w = pool.tile([P, DC, D], dt.float8e4, tag="w8", name=name)
            for kc in range(DC):
                nc.sync.dma_start(
                    out=w[:, kc, :],
                    in_=bass.AP(tensor=dram_h, offset=kc * P * D,
                                ap=[[D, P], [1, D]]),
                )
            return w

        def load_wf(pool, dram_h, name, row0=0, col0=0, ncolw=D):
            """bf16 FFN weight block rows [row0,row0+D) cols [col0,col0+ncolw)
            -> SBUF [P, DC, ncolw]."""
            rows, ncols = dram_h.shape
            w = pool.tile([P, DC, ncolw], dt.bfloat16, tag="wf", name=name)
            for kc in range(DC):
                nc.sync.dma_start(
                    out=w[:, kc, :],
                    in_=bass.AP(tensor=dram_h,
                                offset=(row0 + kc * P) * ncols + col0,
                                ap=[[ncols, P], [1, ncolw]]),
                )
            return w

        def loadT8(pool, dram_h, name):
            t = pool.tile([P, DC, N], dt.float8e4, tag=name, name=name)
            for kc in range(DC):
                nc.sync.dma_start(
                    out=t[:, kc, :],
                    in_=bass.AP(tensor=dram_h, offset=kc * P * N,
                                ap=[[N, P], [1, N]]),
                )
            return t

        def proj_dr(out_sb, w8, inT8, bias_sb, n0=0, nw=N):
            """fp8 DoubleRow projection: out[o, n] = (1/WS)*sum_d W[d,o]x[d,n]
            + b[o], writing out_sb[:, dc, n0:n0+nw] in bf16."""
            for dc in range(DC):
                for nt in range(nw // 512):
                    lo = n0 + nt * 512
                    ps = psp.tile([P, 512], dt.float32, tag="pj")
                    for c in range(DC // 2):
                        nc.tensor.matmul(
                            ps,
                            lhsT=w8[:, 2 * c:2 * c + 2, dc * P:(dc + 1) * P],
                            rhs=inT8[:, 2 * c:2 * c + 2, lo:lo + 512],
                            start=(c == 0), stop=(c == DC // 2 - 1),
                            perf_mode=PM.DoubleRow,
                        )
                    nc.vector.tensor_scalar(
                        out_sb[:, dc, lo:lo + 512],
                        ps, WSI, bias_sb[:, dc:dc + 1], ALU.mult, ALU.add,
                    )

        def proj_v65_dr(v8, w8, inT8):
            """Token-major V projection into fp8 [P, mc, h, 0:64]; col 64=1."""
            nc.gpsimd.memset(v8, 1.0)
            for mc in range(NC_):
                for ot in range(2):
                    ps = psp.tile([P, 512], dt.float32, tag="pj")
                    for c in range(DC // 2):
                        nc.tensor.matmul(
                            ps,
                            lhsT=inT8[:, 2 * c:2 * c + 2, mc * P:(mc + 1) * P],
                            rhs=w8[:, 2 * c:2 * c + 2, ot * 512:(ot + 1) * 512],
                            start=(c == 0), stop=(c == DC // 2 - 1),
                            perf_mode=PM.DoubleRow,
                        )
                    nc.vector.tensor_scalar(
                        v8[:, mc, ot * 8:(ot + 1) * 8, 0:64],
                        ps.rearrange("p (h e) -> p h e", h=8),
                        WSI, None, ALU.mult,
                    )

        def attention_half(qT, kT, v8, o_half, half, tagsuf):
            """o_half[:, la, h*64:(h+1)*64] = softmax-normalized AV for the
            512 tokens of `half`. exp in fp8, AV fp8-DoubleRow over mc pairs,
            2 heads batched per po psum bank."""
            n0 = half * HL
            for hpair in range(H // 2):
                expt = ep.tile([P, 2, NC_, HL], dt.float8e4, tag="exp",
                               name=f"exp_{tagsuf}_{hpair}")
                for h2 in range(2):
                    h_idx = 2 * hpair + h2
                    hp = h_idx // 2
                    lo = (h_idx % 2) * 64
                    for mcp in range(NC_ // 2):
                        sc = psp.tile([P, 2, 512], dt.float32, tag="sc")
                        for j in range(2):
                            mc = 2 * mcp + j
                            nc.tensor.matmul(
                                sc[:, j, :],
                                lhsT=kT[lo:lo + 64, hp, mc * P:(mc + 1) * P],
                                rhs=qT[lo:lo + 64, hp, n0:n0 + HL],
                                start=True, stop=True,
                            )
                        nc.scalar.activation(
                            expt[:, h2, 2 * mcp:2 * mcp + 2, :], sc,
                            AF.Exp, scale=0.125,
                        )
                for la in range(HC):
                    po = psp.tile([P, 2, 65], dt.float32, tag="po")
                    for h2 in range(2):
                        h_idx = 2 * hpair + h2
                        for mcp in range(NC_ // 2):
                            nc.tensor.matmul(
                                po[:, h2, :],
                                lhsT=expt[:, h2, 2 * mcp:2 * mcp + 2,
                                          la * P:(la + 1) * P],
                                rhs=v8[:, 2 * mcp:2 * mcp + 2, h_idx, :],
                                start=(mcp == 0), stop=(mcp == NC_ // 2 - 1),
                                perf_mode=PM.DoubleRow,
                            )
                    rec = small.tile([P, 2], dt.float32, tag="rec")
                    nc.vector.reciprocal(rec, po[:, :, 64:65])
                    for h2 in range(2):
                        h_idx = 2 * hpair + h2
                        nc.vector.tensor_scalar(
                            o_half[:, la, h_idx * 64:(h_idx + 1) * 64],
                            po[:, h2, 0:64],
                            rec[:, h2:h2 + 1], None, ALU.mult,
                        )

        def layernorm_half(pool, o_half, half, x_src, g_d, be_d, dst_sb,
                           xT_out, final=False):
            """dst_sb[:, la, :] = LN(o_half[:, la, :] + x_src_chunk)*g + be
            for the 4 chunks of `half`; optionally also write the
            feature-major transpose into xT_out[:, dc, la*128:...].

            x_src: DRAM handle (LN1), SBUF [P, HC, D] tile (LN2/LN3 residual
            source), or None (residual already in o_half)."""
            mv = small.tile([P, HC, 2], dt.float32, tag="mv")
            for la in range(HC):
                ncc = half * HC + la
                r = o_half[:, la, :]
                if x_src is not None:
                    if hasattr(x_src, "ap") and callable(x_src.ap):  # DRAM
                        xin = pool.tile([P, D], dt.bfloat16, tag="lnx",
                                        bufs=2)
                        nc.sync.dma_start(
                            out=xin, in_=x_src.ap()[ncc * P:(ncc + 1) * P, :])
                    else:
                        xin = x_src[:, la, :]
                    nc.gpsimd.tensor_tensor(out=r, in0=r, in1=xin, op=ALU.add)
                stats = small.tile([P, 2, 6], dt.float32, tag="stats")
                nc.vector.bn_stats(stats[:, 0, :], r[:, 0:512])
                nc.vector.bn_stats(stats[:, 1, :], r[:, 512:1024])
                nc.vector.bn_aggr(mv[:, la, :], stats)
            # rstd = rsqrt(var + eps) via Newton on DVE (keeps ACT exp-only,
            # avoiding activation-table thrash). var is ~1 here (normalized
            # residual stream), so seed 1.0 + 4 iterations converge <1e-4.
            var = small.tile([P, HC], dt.float32, tag="lnv")
            rstd = small.tile([P, HC], dt.float32, tag="rstd")
            nmr = small.tile([P, HC], dt.float32, tag="nmr")
            t_ = small.tile([P, HC], dt.float32, tag="lnt_")
            nc.vector.tensor_scalar(var, mv[:, :, 1], EPS, None, ALU.add)
            nc.vector.tensor_scalar(rstd, var, -0.5, 1.5, ALU.mult, ALU.add)
            for _ in range(4):
                nc.vector.tensor_tensor(out=t_, in0=rstd, in1=rstd,
                                        op=ALU.mult)
                nc.vector.tensor_tensor(out=t_, in0=t_, in1=var, op=ALU.mult)
                nc.vector.tensor_scalar(t_, t_, -0.5, 1.5, ALU.mult, ALU.add)
                nc.vector.tensor_tensor(out=rstd, in0=rstd, in1=t_,
                                        op=ALU.mult)
            nc.vector.tensor_tensor(out=nmr, in0=mv[:, :, 0], in1=rstd,
                                    op=ALU.mult)
            nc.vector.tensor_scalar(nmr, nmr, -1.0, None, ALU.mult)
            if not ln_identity:
                g_t = pool.tile([P, D], dt.bfloat16, tag="lng", bufs=1)
                nc.sync.dma_start(out=g_t, in_=bcast_ap(g_d, D))
                be_t = pool.tile([P, D], dt.bfloat16, tag="lnbe", bufs=1)
                nc.sync.dma_start(out=be_t, in_=bcast_ap(be_d, D))
            for la in range(HC):
                r = o_half[:, la, :]
                if final:
                    tgt = pool.tile([P, D], dt.float32, tag="lnof", bufs=2)
                else:
                    tgt = dst_sb[:, la, :]
                if ln_identity:
                    nc.gpsimd.tensor_scalar(
                        tgt, r, rstd[:, la:la + 1], nmr[:, la:la + 1],
                        ALU.mult, ALU.add)
                else:
                    t = pool.tile([P, D], dt.bfloat16, tag="lnt", bufs=2)
                    nc.gpsimd.tensor_scalar(
                        t, r, rstd[:, la:la + 1], nmr[:, la:la + 1],
                        ALU.mult, ALU.add)
                    nc.gpsimd.tensor_tensor(out=t, in0=t, in1=g_t,
                                            op=ALU.mult)
                    nc.gpsimd.tensor_tensor(out=tgt, in0=t, in1=be_t,
                                            op=ALU.add)
                if final:
                    ncc = half * HC + la
                    nc.sync.dma_start(
                        out=out_d.ap()[ncc * P:(ncc + 1) * P, :], in_=tgt)
                elif xT_out is not None:
                    for dc in range(DC):
                        pst = psp.tile([P, P], dt.bfloat16, tag="po")
                        nc.tensor.transpose(
                            pst, tgt[:, dc * P:(dc + 1) * P], ident_sb)
                        nc.vector.tensor_copy(
                            xT_out[:, dc, la * P:(la + 1) * P], pst)

        def one_layer(rep):
            with ExitStack() as sAB:
                # scope-AB pools, open order fixes the address layout so
                # stage-C pools below reuse the earliest-dying regions first
                w8p = sAB.enter_context(tc.tile_pool(name="w8", bufs=2))
                xp8 = sAB.enter_context(tc.tile_pool(name="x8", bufs=1))
                qk1p = sAB.enter_context(tc.tile_pool(name="qk1", bufs=1))
                v1p = sAB.enter_context(tc.tile_pool(name="v1p", bufs=1))
                lnp1 = sAB.enter_context(tc.tile_pool(name="ln1", bufs=1))
                x1Tp = sAB.enter_context(tc.tile_pool(name="x1T", bufs=1))

                # ---- stage A: inputs + QKV1 ----
                xT8 = loadT8(xp8, xT8_d, "xT8")
                q1T = qk1p.tile([P, DC, N], dt.float8e4, tag="q1T")
                k1T = qk1p.tile([P, DC, N], dt.float8e4, tag="k1T")
                v1_8 = v1p.tile([P, NC_, H, 65], dt.float8e4, tag="v1")
                proj_dr(q1T, load_w8(w8p, w8_d["q1"], f"wq1_{rep}"), xT8,
                        bq1_sb)
                proj_dr(k1T, load_w8(w8p, w8_d["k1"], f"wk1_{rep}"), xT8,
                        bk1_sb)
                proj_v65_dr(v1_8, load_w8(w8p, w8_d["v1"], f"wv1_{rep}"),
                            xT8)

                memT8 = loadT8(xp8, memT8_d, "memT8")
                k2T = qkp.tile([P, DC, N], dt.float8e4, tag="k2T")
                v2_8 = vp.tile([P, NC_, H, 65], dt.float8e4, tag="v2")
                q2T = qkp.tile([P, DC, N], dt.float8e4, tag="q2T")
                wq2 = load_w8(w8p, w8_d["q2"], f"wq2_{rep}")

                # ---- stage B: attention1 + LN1 + Q2, per token half ----
                def ln1_q2(half, o1):
                    x1 = resp.tile([P, HC, D], dt.bfloat16, tag="x1",
                                   name=f"x1_{half}")
                    x1T8 = x1Tp.tile([P, DC, HL], dt.float8e4, tag="x1T8",
                                     name=f"x1T8_{half}", bufs=2)
                    layernorm_half(lnp1, o1, half, x_tok_d, g1_d, be1_d,
                                   x1, x1T8)
                    for dc in range(DC):
                        ps = psp.tile([P, 512], dt.float32, tag="pj")
                        for c in range(DC // 2):
                            nc.tensor.matmul(
                                ps,
                                lhsT=wq2[:, 2 * c:2 * c + 2,
                                         dc * P:(dc + 1) * P],
                                rhs=x1T8[:, 2 * c:2 * c + 2, :],
                                start=(c == 0), stop=(c == DC // 2 - 1),
                                perf_mode=PM.DoubleRow,
                            )
                        nc.vector.tensor_scalar(
                            q2T[:, dc, half * HL:(half + 1) * HL],
                            ps, WSI, bq2_sb[:, dc:dc + 1], ALU.mult, ALU.add,
                        )
                    return x1

                x1h = [None, None]
                o1 = op_.tile([P, HC, D], dt.bfloat16, tag="o", name="o1_0")
                attention_half(q1T, k1T, v1_8, o1, 0, f"a1h0_{rep}")
                x1h[0] = ln1_q2(0, o1)
                o1 = op_.tile([P, HC, D], dt.bfloat16, tag="o", name="o1_1")
                attention_half(q1T, k1T, v1_8, o1, 1, f"a1h1_{rep}")
                # cross-attention K/V fill attention1's PE gaps
                proj_dr(k2T, load_w8(w8p, w8_d["k2"], f"wk2_{rep}"),
                        memT8, bk2_sb)
                proj_v65_dr(v2_8, load_w8(w8p, w8_d["v2"], f"wv2_{rep}"),
                            memT8)
                x1h[1] = ln1_q2(1, o1)

            # ---- stage C: attention2 + LN2 + FFN + LN3, per token half ----
            with ExitStack() as sC:
                wfp = sC.enter_context(tc.tile_pool(name="wf", bufs=2))
                hp_ = sC.enter_context(tc.tile_pool(name="hp", bufs=2))
                yp = sC.enter_context(tc.tile_pool(name="yp", bufs=2))
                x2Tp = sC.enter_context(tc.tile_pool(name="x2T", bufs=1))
                lnp2 = sC.enter_context(tc.tile_pool(name="ln2", bufs=1))

                x2hs, x2Ts = [], []
                for half in range(NH):
                    o2 = op_.tile([P, HC, D], dt.bfloat16, tag="o",
                                  name=f"o2_{half}")
                    attention_half(q2T, k2T, v2_8, o2, half,
                                   f"a2h{half}_{rep}")
                    x2h = resp.tile([P, HC, D], dt.bfloat16, tag="x2",
                                    name=f"x2_{half}")
                    x2T = x2Tp.tile([P, DC, HL], dt.bfloat16, tag="x2T",
                                    name=f"x2T_{half}", bufs=2)
                    layernorm_half(lnp2, o2, half, x1h[half], g2_d, be2_d,
                                   x2h, x2T)
                    x2hs.append(x2h)
                    x2Ts.append(x2T)
                for half in range(NH):
                    x2h, x2T = x2hs[half], x2Ts[half]
                    y = yp.tile([P, HC, D], dt.float32, tag="y",
                                name=f"y_{half}")
                    for f in range(FF // D):
                        w1f = load_wf(wfp, W1T_d, f"w1_{f}_{half}_{rep}",
                                      col0=f * D)
                        hf = hp_.tile([P, DC, HL], dt.bfloat16, tag="h")
                        for fc in range(DC):
                            ps = psp.tile([P, 512], dt.float32, tag="pj")
                            for kc in range(DC):
                                nc.tensor.matmul(
                                    ps,
                                    lhsT=w1f[:, kc, fc * P:(fc + 1) * P],
                                    rhs=x2T[:, kc, :],
                                    start=(kc == 0), stop=(kc == DC - 1),
                                )
                            nc.vector.tensor_scalar(
                                hf[:, fc, :],
                                ps, b1_sb[:, f * DC + fc:f * DC + fc + 1],
                                0.0, ALU.add, ALU.max,
                            )
                        w2f = load_wf(wfp, W2T_d, f"w2_{f}_{half}_{rep}",
                                      row0=f * D)
                        for la in range(HC):
                            for dtile in range(2):
                                ps = psp.tile([P, 512], dt.float32, tag="pj")
                                for fc in range(DC):
                                    nc.tensor.matmul(
                                        ps,
                                        lhsT=hf[:, fc, la * P:(la + 1) * P],
                                        rhs=w2f[:, fc,
                                                dtile * 512:(dtile + 1) * 512],
                                        start=(fc == 0), stop=(fc == DC - 1),
                                    )
                                ysl = y[:, la, dtile * 512:(dtile + 1) * 512]
                                if f == 0:
                                    nc.vector.scalar_tensor_tensor(
                                        out=ysl, in0=ps, scalar=1.0,
                                        in1=x2h[:, la,
                                                dtile * 512:(dtile + 1) * 512],
                                        op0=ALU.mult, op1=ALU.add,
                                    )
                                else:
                                    nc.vector.tensor_tensor(
                                        out=ysl, in0=ps, in1=ysl, op=ALU.add)
                    layernorm_half(lnp2, y, half, None, g3_d, be3_d, None,
                                   None, final=True)

        for _rep in range(loop_n):
            one_layer(_rep)

    nc.compile()
    _BUILD_CACHE[key] = nc
    return nc


def _prep_inputs(inputs):
    """Host-side prep: transposes, bf16/fp8 casts, bias folding."""
    f32 = np.float32

    def t_bf16(a):
        return np.ascontiguousarray(np.asarray(a, dtype=f32).T).astype(BF16)

    def t_fp8(a, scale=1.0):
        return np.ascontiguousarray(
            np.asarray(a, dtype=f32).T * scale).astype(FP8)

    x = np.asarray(inputs["x"], dtype=f32)
    memory = np.asarray(inputs["memory"], dtype=f32)
    Wq2 = np.asarray(inputs["Wq2"], dtype=f32)
    W1 = np.asarray(inputs["W1"], dtype=f32)
    bq2 = np.asarray(inputs["bq2"], dtype=f32)
    bv1 = np.asarray(inputs["bv1"], dtype=f32)
    bv2 = np.asarray(inputs["bv2"], dtype=f32)
    b1 = np.asarray(inputs["b1"], dtype=f32)
    b2 = np.asarray(inputs["b2"], dtype=f32)
    g1 = np.asarray(inputs["g1"], dtype=f32)
    be1 = np.asarray(inputs["be1"], dtype=f32)
    g2 = np.asarray(inputs["g2"], dtype=f32)
    be2 = np.asarray(inputs["be2"], dtype=f32)
    g3 = np.asarray(inputs["g3"], dtype=f32)
    be3 = np.asarray(inputs["be3"], dtype=f32)

    be1p = (be1 + bv2).astype(f32)
    be2p = (be2 + b2).astype(f32)
    ln_identity = bool(
        np.all(g1 == 1) and np.all(g2 == 1) and np.all(g3 == 1)
        and np.all(be1p == 0) and np.all(be2p == 0) and np.all(be3 == 0)
    )

    shared = {
        "Wq1T8": t_fp8(inputs["Wq1"], WS), "Wk1T8": t_fp8(inputs["Wk1"], WS),
        "Wv1T8": t_fp8(inputs["Wv1"], WS), "Wq2T8": t_fp8(Wq2, WS),
        "Wk2T8": t_fp8(inputs["Wk2"], WS), "Wv2T8": t_fp8(inputs["Wv2"], WS),
        "W1T": t_bf16(W1), "W2T": t_bf16(inputs["W2"]),
        "bq1": np.asarray(inputs["bq1"], f32),
        "bk1": np.asarray(inputs["bk1"], f32),
        "bq2p": (bq2 - Wq2 @ bv2).astype(f32),
        "bk2": np.asarray(inputs["bk2"], f32),
        "b1p": (b1 - W1 @ b2).astype(f32),
        "g1": g1.astype(BF16), "be1p": be1p.astype(BF16),
        "g2": g2.astype(BF16), "be2p": be2p.astype(BF16),
        "g3": g3.astype(BF16), "be3": be3.astype(BF16),
    }

    in_maps = []
    for i in range(N_CORES):
        m = dict(shared)
        m["x_tok"] = (x[i] + bv1[None, :]).astype(BF16)
        m["xT8"] = t_fp8(x[i])
        m["memT8"] = t_fp8(memory[i])
        in_maps.append(m)
    return in_maps, ln_identity


def kernel(**inputs) -> np.ndarray:
    os.environ.setdefault("MYCRO_LOCAL_CACHE", "1")
    from concourse.bass_utils import run_bass_kernel_spmd

    in_maps, ln_identity = _prep_inputs(inputs)
    nc = _build_program(1, ln_identity)
    res = run_bass_kernel_spmd(nc, in_maps, core_ids=list(range(N_CORES)))
    out = np.stack([res.results[i]["out"] for i in range(N_CORES)], axis=0)
    return out.astype(np.float32)
